# revision 4
# baseline (speedup 1.0000x reference)
"""Trainium2 Bass kernel for nn_Custom_FeedForward_75393855914399.

Quantized SwiGLU MLP: S=1024, E=4096, H=11008, int8 GEMMs (values stored
int32) with per-row requantization, int8 sigmoid / silu / mul, and a final
int8 GEMM.

Sharding (8 cores, tensor-parallel Megatron split on H):
  - fc1/fc2 column-parallel: core c owns H rows [c*1376, (c+1)*1376) of
    W1/W2 (padded to 1408 with zeros; padded columns produce mul_q == 0
    exactly, so they are harmless in fc3).
  - the whole quantized SwiGLU elementwise chain runs on the local H slice
    in s-major layout (per-row scales become cheap per-partition scalars).
  - mul_q (int8-valued, carried as bf16) is AllGathered: 2.75 MB/rank
    instead of all-reducing the 16.8 MB fc3 output.
  - fc3 is then column-parallel over E: each core contracts the FULL H
    against its 512 output columns of W3 -> no reduction needed at all.
  - host concatenates the 8 [512, 1024] outputs and transposes.

All matmuls run in bf16 (int8 values are exact in bf16; products and the
fp32 PSUM accumulation stay exact integers far below 2^24), so results
are bit-identical to the int32-accumulate reference except for the
sigmoid spline (<= 40 ULP) and scale-association differences at exact
rounding boundaries (~1e-4 of elements off by one quantization step).

Round-to-nearest-even is implemented with the fp32 magic constant
(x + 1.5*2^23) - 1.5*2^23, matching jnp.round for |x| < 2^22.
"""

import numpy as np
import ml_dtypes

import concourse.bass as bass
import concourse.mybir as mybir
import concourse.tile as tile
from concourse import bacc
from concourse.bass_utils import run_bass_kernel_spmd

F32 = mybir.dt.float32
BF16 = mybir.dt.bfloat16
MAGIC = float(1.5 * 2**23)  # 12582912.0 — fp32 RNE-to-integer magic
P = 128


class Cfg:
    def __init__(self, S, E, H, n_cores):
        self.S, self.E, self.H, self.n_cores = S, E, H, n_cores
        assert H % n_cores == 0
        self.H_LOC = H // n_cores                      # true per-core H slice
        self.H_PAD = ((self.H_LOC + P - 1) // P) * P   # padded to 128
        assert self.H_PAD % 4 == 0
        self.QW = self.H_PAD // 4                      # weight-quarter width
        assert E % (n_cores * P) == 0
        self.E_LOC = E // n_cores                      # fc3 output cols per core
        self.E_T = E // P                              # contraction tiles fc1/2
        self.S_T = S // P
        self.ET_LOC = self.E_LOC // P
        self.HK = n_cores * self.H_PAD // P            # fc3 contraction tiles
        # s chunks of <=512 for matmul moving free dim / psum banks
        self.SC = [(i, min(512, S - i)) for i in range(0, S, 512)]


FULL = Cfg(S=1024, E=4096, H=11008, n_cores=8)


def build_kernel(cfg):
    """Build + compile the SPMD Bass module (same code on every core)."""
    nc = bacc.Bacc(
        "TRN2",
        target_bir_lowering=False,
        debug=False,
        enable_asserts=False,
        num_devices=cfg.n_cores,
    )
    S, E_T, S_T, QW, H_PAD = cfg.S, cfg.E_T, cfg.S_T, cfg.QW, cfg.H_PAD

    xt = nc.dram_tensor("xt", [cfg.E, S], BF16, kind="ExternalInput")
    w1t = nc.dram_tensor("w1t", [cfg.E, H_PAD], BF16, kind="ExternalInput")
    w2t = nc.dram_tensor("w2t", [cfg.E, H_PAD], BF16, kind="ExternalInput")
    w3t = nc.dram_tensor("w3t", [cfg.HK * P, cfg.E_LOC], BF16, kind="ExternalInput")
    # per-row scale vectors, packed [P, 6, S_T]:
    #   0: A1  = scale_x*scale_w1/scale_y1
    #   1: A2  = scale_x*scale_w2/scale_y2
    #   2: SY1 = scale_y1
    #   3: CI  = 1/scale_sig
    #   4: EP  = scale_y1*scale_sig/scale_silu_mul
    #   5: GP  = scale_silu_mul*scale_y2/scale_mul
    scales = nc.dram_tensor("scales", [P, 6, S_T], F32, kind="ExternalInput")
    # A3 = scale_mul*scale_w3/scale_y3 broadcast across partitions [P, S]
    a3bc = nc.dram_tensor("a3bc", [P, S], F32, kind="ExternalInput")
    out = nc.dram_tensor("out", [cfg.E_LOC, S], F32, kind="ExternalOutput")

    # internal DRAM: local mul_q block and its AllGather across cores
    mulq_local = nc.dram_tensor("mulq_local", [S, H_PAD], BF16)
    mulq_all = nc.dram_tensor(
        "mulq_all", [cfg.n_cores * S, H_PAD], BF16, addr_space="Shared"
    )

    xt_r = xt.ap().rearrange("(o p) s -> p o s", p=P)
    w1_r = w1t.ap().rearrange("(o p) h -> p o h", p=P)
    w2_r = w2t.ap().rearrange("(o p) h -> p o h", p=P)

    TS = mybir.AluOpType
    ACTF = mybir.ActivationFunctionType

    with tile.TileContext(nc) as tc:
        with (
            tc.tile_pool(name="const", bufs=1) as cpool,
            tc.tile_pool(name="wq", bufs=2) as wpool,
            tc.tile_pool(name="ew", bufs=2) as ew,
            tc.tile_pool(name="mq", bufs=1) as mqpool,
            tc.tile_pool(name="psb", bufs=2, space="PSUM") as pp,
        ):
            # ---- resident tensors ----
            xt_sb = cpool.tile([P, E_T, S], BF16)
            for g in range(0, E_T, 4):
                nc.sync.dma_start(xt_sb[:, g : g + 4, :], xt_r[:, g : g + 4, :])
            sc_sb = cpool.tile([P, 6, S_T], F32)
            nc.sync.dma_start(sc_sb[:], scales[:, :, :])
            a3_sb = cpool.tile([P, S], F32)
            nc.sync.dma_start(a3_sb[:], a3bc[:, :])
            mg = cpool.tile([P, 1], F32)
            nc.vector.memset(mg[:], MAGIC)

            mq_all = mqpool.tile([P, S_T, H_PAD], BF16)

            # ---- phase B: fc1/fc2 + quantized SwiGLU chain ----
            for q in range(4):
                h0 = q * QW
                w1q = wpool.tile([P, E_T, QW], BF16, tag="w1q")
                w2q = wpool.tile([P, E_T, QW], BF16, tag="w2q")
                for g in range(0, E_T, 4):
                    nc.sync.dma_start(
                        w1q[:, g : g + 4, :], w1_r[:, g : g + 4, h0 : h0 + QW]
                    )
                    nc.sync.dma_start(
                        w2q[:, g : g + 4, :], w2_r[:, g : g + 4, h0 : h0 + QW]
                    )
                for st in range(S_T):
                    psA = pp.tile([P, QW], F32, tag="psA")
                    psB = pp.tile([P, QW], F32, tag="psB")
                    for ek in range(E_T):
                        lhs = xt_sb[:, ek, st * P : (st + 1) * P]
                        first, last = ek == 0, ek == E_T - 1
                        nc.tensor.matmul(
                            psA[:], lhs, w1q[:, ek, :], start=first, stop=last
                        )
                        nc.tensor.matmul(
                            psB[:], lhs, w2q[:, ek, :], start=first, stop=last
                        )

                    a1v = sc_sb[:, 0, st : st + 1]
                    a2v = sc_sb[:, 1, st : st + 1]
                    sy1v = sc_sb[:, 2, st : st + 1]
                    civ = sc_sb[:, 3, st : st + 1]
                    epv = sc_sb[:, 4, st : st + 1]
                    gpv = sc_sb[:, 5, st : st + 1]

                    ewt = ew.tile([P, 5, QW], F32, tag="ewt")
                    fc1q = ewt[:, 0, :]
                    fc2q = ewt[:, 1, :]
                    sg = ewt[:, 2, :]
                    t = ewt[:, 3, :]
                    u = ewt[:, 4, :]

                    # fc1_q = q8(acc1 * A1)   (DVE, psum source)
                    nc.vector.tensor_scalar(fc1q, psA[:], a1v, MAGIC, TS.mult, TS.add)
                    nc.vector.tensor_scalar(
                        fc1q, fc1q, MAGIC, 127.0, TS.subtract, TS.min
                    )
                    nc.vector.tensor_scalar_max(fc1q, fc1q, -128.0)
                    # fc2_q = q8(acc2 * A2)   (ACT evacuates psum, then DVE clips)
                    nc.scalar.activation(
                        fc2q, psB[:], ACTF.Identity, bias=mg[:, 0:1], scale=a2v
                    )
                    nc.vector.tensor_scalar(
                        fc2q, fc2q, MAGIC, 127.0, TS.subtract, TS.min
                    )
                    nc.vector.tensor_scalar_max(fc2q, fc2q, -128.0)
                    # sig_q = q8(sigmoid(fc1_q * scale_y1) / scale_sig); >= 0
                    nc.scalar.activation(sg, fc1q, ACTF.Sigmoid, scale=sy1v)
                    nc.scalar.activation(sg, sg, ACTF.Identity, bias=mg[:, 0:1], scale=civ)
                    nc.vector.tensor_scalar(sg, sg, MAGIC, 127.0, TS.subtract, TS.min)
                    # silu_q = q8(fc1_q * sig_q * EP)
                    nc.vector.tensor_tensor(t, fc1q, sg, TS.mult)
                    nc.vector.tensor_scalar(t, t, epv, MAGIC, TS.mult, TS.add)
                    nc.vector.tensor_scalar(t, t, MAGIC, 127.0, TS.subtract, TS.min)
                    nc.vector.tensor_scalar_max(t, t, -128.0)
                    # mul_q = q8(silu_q * fc2_q * GP) -> bf16 (exact int8 values)
                    nc.vector.tensor_tensor(u, t, fc2q, TS.mult)
                    nc.vector.tensor_scalar(u, u, gpv, MAGIC, TS.mult, TS.add)
                    nc.vector.tensor_scalar(u, u, MAGIC, 127.0, TS.subtract, TS.min)
                    nc.vector.tensor_scalar_max(
                        mq_all[:, st, h0 : h0 + QW], u, -128.0
                    )

            for st in range(S_T):
                nc.sync.dma_start(
                    mulq_local[st * P : (st + 1) * P, :], mq_all[:, st, :]
                )

            # ---- phase C: AllGather mul_q across the 8 cores ----
            if cfg.n_cores > 1:
                nc.gpsimd.collective_compute(
                    "AllGather",
                    TS.bypass,
                    replica_groups=[list(range(cfg.n_cores))],
                    ins=[mulq_local.ap().opt()],
                    outs=[mulq_all.ap().opt()],
                )
                mq_src = mulq_all
            else:
                mq_src = mulq_local

        # ---- phase D: fc3 over full H, E_LOC output columns ----
        with (
            tc.tile_pool(name="f3", bufs=3) as f3,
            tc.tile_pool(name="outp", bufs=2) as outp,
            tc.tile_pool(name="ps3", bufs=1, space="PSUM") as pp3,
        ):
            nsc = len(cfg.SC)
            ps3 = [
                [
                    pp3.tile([P, w], F32, tag=f"o_{et}_{si}", name=f"o_{et}_{si}")
                    for si, (s0, w) in enumerate(cfg.SC)
                ]
                for et in range(cfg.ET_LOC)
            ]
            jmax = H_PAD // P
            for hk in range(cfg.HK):
                blk, j = divmod(hk, jmax)
                mqt = f3.tile([P, S], BF16, tag="mqt")
                nc.sync.dma_start_transpose(
                    mqt[:],
                    mq_src[blk * S : (blk + 1) * S, j * P : (j + 1) * P],
                )
                w3q = f3.tile([P, cfg.E_LOC], BF16, tag="w3q")
                nc.sync.dma_start(w3q[:], w3t[hk * P : (hk + 1) * P, :])
                first, last = hk == 0, hk == cfg.HK - 1
                for et in range(cfg.ET_LOC):
                    for si, (s0, w) in enumerate(cfg.SC):
                        nc.tensor.matmul(
                            ps3[et][si][:],
                            w3q[:, et * P : (et + 1) * P],
                            mqt[:, s0 : s0 + w],
                            start=first,
                            stop=last,
                        )
            # out_q = q8(acc3 * A3)
            for et in range(cfg.ET_LOC):
                ot = outp.tile([P, S], F32, tag="ot")
                for si, (s0, w) in enumerate(cfg.SC):
                    nc.vector.tensor_tensor(
                        ot[:, s0 : s0 + w], ps3[et][si][:], a3_sb[:, s0 : s0 + w],
                        TS.mult,
                    )
                nc.vector.tensor_scalar(ot, ot, MAGIC, MAGIC, TS.add, TS.subtract)
                nc.vector.tensor_scalar(ot, ot, 127.0, -128.0, TS.min, TS.max)
                nc.sync.dma_start(out[et * P : (et + 1) * P, :], ot)

    nc.compile()
    return nc


def prep_in_maps(cfg, x, W1, W2, W3, scale_x, scale_w1, scale_w2, scale_w3,
                 scale_y1, scale_y2, scale_sig, scale_silu_mul, scale_mul,
                 scale_y3):
    """Host-side sharding/layout: transpose + cast + pad + scale folding."""
    bf16 = ml_dtypes.bfloat16
    f32 = np.float32
    S, H_LOC, H_PAD, NC = cfg.S, cfg.H_LOC, cfg.H_PAD, cfg.n_cores

    def v32(a):
        return np.asarray(a, f32)

    scale_x, scale_y1, scale_y2 = v32(scale_x), v32(scale_y1), v32(scale_y2)
    scale_sig, scale_silu_mul = v32(scale_sig), v32(scale_silu_mul)
    scale_mul, scale_y3 = v32(scale_mul), v32(scale_y3)
    scale_w1, scale_w2, scale_w3 = f32(scale_w1), f32(scale_w2), f32(scale_w3)

    A1 = scale_x * scale_w1 / scale_y1
    A2 = scale_x * scale_w2 / scale_y2
    SY1 = scale_y1
    CI = f32(1.0) / scale_sig
    EP = scale_y1 * scale_sig / scale_silu_mul
    GP = scale_silu_mul * scale_y2 / scale_mul
    A3 = scale_mul * scale_w3 / scale_y3

    sc_pack = np.stack(
        [v.reshape(cfg.S_T, P).T for v in (A1, A2, SY1, CI, EP, GP)], axis=1
    )  # [P, 6, S_T]
    sc_pack = np.ascontiguousarray(sc_pack, dtype=f32)
    a3bc = np.ascontiguousarray(np.broadcast_to(A3[None, :], (P, S)), dtype=f32)

    xt = np.ascontiguousarray(np.asarray(x, np.int32).T).astype(f32).astype(bf16)

    W1 = np.asarray(W1, np.int32)
    W2 = np.asarray(W2, np.int32)
    W3 = np.asarray(W3, np.int32)

    in_maps = []
    for c in range(NC):
        w1t = np.zeros((cfg.E, H_PAD), dtype=bf16)
        w2t = np.zeros((cfg.E, H_PAD), dtype=bf16)
        rows = slice(c * H_LOC, (c + 1) * H_LOC)
        w1t[:, :H_LOC] = W1[rows, :].T.astype(f32).astype(bf16)
        w2t[:, :H_LOC] = W2[rows, :].T.astype(f32).astype(bf16)
        w3c = W3[c * cfg.E_LOC : (c + 1) * cfg.E_LOC, :]  # [E_LOC, H]
        w3t = np.zeros((NC, H_PAD, cfg.E_LOC), dtype=bf16)
        for r in range(NC):
            w3t[r, :H_LOC, :] = (
                w3c[:, r * H_LOC : (r + 1) * H_LOC].T.astype(f32).astype(bf16)
            )
        in_maps.append(
            {
                "xt": xt,
                "w1t": w1t,
                "w2t": w2t,
                "w3t": np.ascontiguousarray(w3t.reshape(NC * H_PAD, cfg.E_LOC)),
                "scales": sc_pack,
                "a3bc": a3bc,
            }
        )
    return in_maps


_CACHE = {}


def _get_nc(cfg):
    key = (cfg.S, cfg.E, cfg.H, cfg.n_cores)
    if key not in _CACHE:
        _CACHE[key] = build_kernel(cfg)
    return _CACHE[key]


def kernel(**inputs):
    cfg = FULL
    nc = _get_nc(cfg)
    in_maps = prep_in_maps(cfg, **inputs)
    res = run_bass_kernel_spmd(nc, in_maps, core_ids=list(range(cfg.n_cores)))
    out_full = np.concatenate(
        [res.results[c]["out"] for c in range(cfg.n_cores)], axis=0
    )  # [E, S]
    out_q = np.ascontiguousarray(out_full.T).astype(np.float32)  # [S, E]
    return out_q, np.asarray(inputs["scale_y3"], np.float32)
